# revision 10
# baseline (speedup 1.0000x reference)
"""Bahdanau-attention kernel for Trainium2 (8 NeuronCores, SPMD data parallel).

Math: the reference's per-step softmax is over a singleton axis, so the
attention weights are exactly 1.0. Hence:
    context  = values.sum(axis=1)            [B, DV]
    attn     = ones(B, T, 1)
    coverage[b, t, 0] = t                    [B, T, 1]
The W1/W2/W3/V MLP cancels out of every output.

Device work: per core, reduce a [B/8, T, DV] shard of `values` over T.
All chunks stream in via plain HWDGE loads (these saturate the fabric);
the fp32 adds are split across two engines so neither exceeds the DMA
roofline:
  - early ND chunks per batch: DVE tensor_add chain into dacc
  - late NP chunks per batch: direct PE matmul vs a ones column, with the
    dacc contraction folded into the same PSUM accumulation group.
attn/coverage come from a tiny host const tensor, written out by DMA.
"""

import os
import numpy as np

B, T, DV = 32, 2048, 1024
NCORES = 8
BP = B // NCORES          # 4 batches per core
TCH = 128                 # t-chunk rows = SBUF partitions
NCH = T // TCH            # 16 chunks of [128, DV] per batch
NSPLIT = 512              # PSUM bank free-dim limit (f32)
NJ = DV // NSPLIT         # 2 psum column groups

ND = 10                   # chunks per batch summed on DVE (chunks 0..ND-1)
NP = NCH - ND             # chunks per batch contracted directly on PE

_CACHE = {}
LAST = {}                 # exec_time_ns etc. for the test harness


def _build_nc():
    import concourse.tile as tile
    from concourse import bacc, mybir
    from contextlib import ExitStack

    f32 = mybir.dt.float32
    nc = bacc.Bacc(
        "TRN2", target_bir_lowering=False, debug=False, num_devices=NCORES
    )

    vals = nc.dram_tensor("vals", [BP, T, DV], f32, kind="ExternalInput").ap()
    consts = nc.dram_tensor("consts", [2, T], f32, kind="ExternalInput").ap()
    ctx_out = nc.dram_tensor("ctx_out", [BP, DV], f32, kind="ExternalOutput").ap()
    attn_out = nc.dram_tensor("attn_out", [BP, T, 1], f32, kind="ExternalOutput").ap()
    cov_out = nc.dram_tensor("cov_out", [BP, T, 1], f32, kind="ExternalOutput").ap()

    with tile.TileContext(nc) as tc, ExitStack() as ctx:
        cpool = ctx.enter_context(tc.tile_pool(name="const", bufs=1))
        vpool = ctx.enter_context(tc.tile_pool(name="vals", bufs=20))
        dpool = ctx.enter_context(tc.tile_pool(name="dacc", bufs=1))
        ppool = ctx.enter_context(tc.tile_pool(name="ps", bufs=1, space="PSUM"))
        opool = ctx.enter_context(tc.tile_pool(name="out", bufs=2))

        ones_t = cpool.tile([128, 1], f32)
        nc.vector.memset(ones_t[:], 1.0)

        const_t = cpool.tile([2, T], f32)
        nc.sync.dma_start(out=const_t[:], in_=consts[:])

        for b in range(BP):
            # --- DVE chunks 0..ND-1: load then chain-add into dacc ---
            dts = []
            for k in range(ND):
                dt_ = vpool.tile([TCH, DV], f32, name=f"dt{b}_{k}", tag="vt")
                nc.sync.dma_start(
                    out=dt_[:], in_=vals[b, k * TCH:(k + 1) * TCH, :])
                dts.append(dt_)
            dacc = dpool.tile([TCH, DV], f32, name=f"dacc{b}", tag=f"dacc{b}")
            nc.vector.tensor_add(dacc[:], dts[0][:], dts[1][:])
            for dt_ in dts[2:]:
                nc.vector.tensor_add(dacc[:], dacc[:], dt_[:])

            # --- PE chunks ND..15: load, matmul directly into psum group ---
            vts = []
            for k in range(ND, NCH):
                vt = vpool.tile([TCH, DV], f32, name=f"vt{b}_{k}", tag="vt")
                nc.sync.dma_start(
                    out=vt[:], in_=vals[b, k * TCH:(k + 1) * TCH, :])
                vts.append(vt)

            # group order per (b, j): first half of direct chunks, then the
            # dacc contraction (its DVE chain finishes around mid-stream),
            # then the late chunks; stop on the last-arriving chunk.
            ps = [
                ppool.tile([1, NSPLIT], f32, name=f"ps{b}_{j}", tag=f"ps{b}_{j}")
                for j in range(NJ)
            ]
            # j-inner pairs: each chunk is consumed by both column groups
            # back-to-back so its buffer slot frees immediately.
            nmm = NP + 1
            order = vts[:NP // 2] + [dacc] + vts[NP // 2:]
            for i, src in enumerate(order):
                for j in range(NJ):
                    sl = slice(j * NSPLIT, (j + 1) * NSPLIT)
                    nc.tensor.matmul(
                        ps[j][:], ones_t[:], src[:, sl],
                        start=(i == 0), stop=(i == nmm - 1))

            ot = opool.tile([1, DV], f32, name=f"ot{b}", tag="ot")
            for j in range(NJ):
                nc.scalar.copy(ot[:, j * NSPLIT:(j + 1) * NSPLIT], ps[j][:])
            nc.sync.dma_start(out=ctx_out[b:b + 1, :], in_=ot[0:1, :])

        for b in range(BP):
            nc.sync.dma_start(out=attn_out[b:b + 1, :, 0], in_=const_t[0:1, :])
            nc.sync.dma_start(out=cov_out[b:b + 1, :, 0], in_=const_t[1:2, :])

    nc.compile()
    return nc


def kernel(query=None, values=None, **unused_weights):
    from concourse.bass_utils import run_bass_kernel_spmd

    values = np.ascontiguousarray(np.asarray(values, dtype=np.float32))
    assert values.shape == (B, T, DV), values.shape

    if "nc" not in _CACHE:
        _CACHE["nc"] = _build_nc()
    nc = _CACHE["nc"]

    consts = np.stack(
        [np.ones(T, dtype=np.float32), np.arange(T, dtype=np.float32)]
    )
    core_ids = list(range(NCORES))
    in_maps = [
        {"vals": values[c * BP:(c + 1) * BP], "consts": consts}
        for c in core_ids
    ]

    trace = bool(int(os.environ.get("BASS_KERNEL_TRACE", "0")))
    res = run_bass_kernel_spmd(nc, in_maps, core_ids, trace=trace)
    LAST["exec_time_ns"] = res.exec_time_ns
    LAST["results"] = res

    context = np.concatenate([res.results[c]["ctx_out"] for c in core_ids], axis=0)
    attn = np.concatenate([res.results[c]["attn_out"] for c in core_ids], axis=0)
    coverage = np.concatenate([res.results[c]["cov_out"] for c in core_ids], axis=0)
    return context, attn, coverage


# revision 14
# speedup vs baseline: 1.0484x; 1.0484x over previous
"""Bahdanau-attention kernel for Trainium2 (8 NeuronCores, SPMD data parallel).

Math: the reference's per-step softmax is over a singleton axis, so the
attention weights are exactly 1.0. Hence:
    context  = values.sum(axis=1)            [B, DV]
    attn     = ones(B, T, 1)
    coverage[b, t, 0] = t                    [B, T, 1]
The W1/W2/W3/V MLP cancels out of every output.

Device work: per core, reduce a [B/8, T, DV] shard of `values` over T.
All chunks stream in via plain HWDGE loads; the fp32 adds are split across
DVE (most chunks, serial add chains) and PE (every 4th chunk, direct
matmul vs a ones column into the PSUM accumulation group). The dacc
contraction optionally goes through the single-pass float32r matmul path.
attn/coverage come from a tiny host const tensor, written out by DMA.
"""

import os
import numpy as np

B, T, DV = 32, 2048, 1024
NCORES = 8
BP = B // NCORES          # 4 batches per core
TCH = 128                 # t-chunk rows = SBUF partitions
NCH = T // TCH            # 16 chunks of [128, DV] per batch
NSPLIT = 512              # PSUM bank free-dim limit (f32)
NJ = DV // NSPLIT         # 2 psum column groups

PE_EVERY = 4              # chunks k with k % PE_EVERY == PE_PHASE go to PE
PE_PHASE = 2
FP32R_DACC = True         # contract dacc via single-pass float32r matmul

_CACHE = {}
LAST = {}                 # exec_time_ns etc. for the test harness


def _build_nc():
    import concourse.tile as tile
    from concourse import bacc, mybir
    from contextlib import ExitStack

    f32 = mybir.dt.float32
    f32r = mybir.dt.float32r
    nc = bacc.Bacc(
        "TRN2", target_bir_lowering=False, debug=False, num_devices=NCORES
    )

    vals = nc.dram_tensor("vals", [BP, T, DV], f32, kind="ExternalInput").ap()
    consts = nc.dram_tensor("consts", [2, T], f32, kind="ExternalInput").ap()
    ctx_out = nc.dram_tensor("ctx_out", [BP, DV], f32, kind="ExternalOutput").ap()
    attn_out = nc.dram_tensor("attn_out", [BP, T, 1], f32, kind="ExternalOutput").ap()
    cov_out = nc.dram_tensor("cov_out", [BP, T, 1], f32, kind="ExternalOutput").ap()

    pe_chunks = [k for k in range(NCH) if k % PE_EVERY == PE_PHASE]
    dv_chunks = [k for k in range(NCH) if k % PE_EVERY != PE_PHASE]

    with tile.TileContext(nc) as tc, ExitStack() as ctx:
        cpool = ctx.enter_context(tc.tile_pool(name="const", bufs=1))
        vpool = ctx.enter_context(tc.tile_pool(name="vals", bufs=20))
        dpool = ctx.enter_context(tc.tile_pool(name="dacc", bufs=1))
        ppool = ctx.enter_context(tc.tile_pool(name="ps", bufs=1, space="PSUM"))
        opool = ctx.enter_context(tc.tile_pool(name="out", bufs=2))

        ones_t = cpool.tile([128, 1], f32)
        nc.vector.memset(ones_t[:], 1.0)
        if FP32R_DACC:
            ones_r = cpool.tile([128, 1], f32r)
            nc.vector.tensor_copy(ones_r[:], ones_t[:])

        const_t = cpool.tile([2, T], f32)
        nc.sync.dma_start(out=const_t[:], in_=consts[:])

        for b in range(BP):
            tiles = {}
            dacc = dpool.tile([TCH, DV], f32, name=f"dacc{b}", tag=f"dacc{b}")
            ndv = 0
            for k in range(NCH):
                vt = vpool.tile([TCH, DV], f32, name=f"vt{b}_{k}", tag="vt")
                nc.sync.dma_start(
                    out=vt[:], in_=vals[b, k * TCH:(k + 1) * TCH, :])
                if k in pe_chunks:
                    tiles[k] = vt
                else:
                    ndv += 1
                    if ndv == 1:
                        first = vt
                    elif ndv == 2:
                        nc.vector.tensor_add(dacc[:], first[:], vt[:])
                    else:
                        nc.vector.tensor_add(dacc[:], dacc[:], vt[:])

            if FP32R_DACC:
                daccr = dpool.tile(
                    [TCH, DV], f32r, name=f"daccr{b}", tag=f"daccr{b}")
                nc.vector.tensor_copy(daccr[:], dacc[:])
                dacc_mm, ones_mm = daccr, ones_r
            else:
                dacc_mm, ones_mm = dacc, ones_t

            ps = [
                ppool.tile([1, NSPLIT], f32, name=f"ps{b}_{j}", tag=f"ps{b}_{j}")
                for j in range(NJ)
            ]
            nmm = len(pe_chunks) + 1
            for i, k in enumerate(pe_chunks):
                for j in range(NJ):
                    sl = slice(j * NSPLIT, (j + 1) * NSPLIT)
                    nc.tensor.matmul(
                        ps[j][:], ones_t[:], tiles[k][:, sl],
                        start=(i == 0), stop=False)
            for j in range(NJ):
                sl = slice(j * NSPLIT, (j + 1) * NSPLIT)
                nc.tensor.matmul(
                    ps[j][:], ones_mm[:], dacc_mm[:, sl],
                    start=False, stop=True)

            ot = opool.tile([1, DV], f32, name=f"ot{b}", tag="ot")
            for j in range(NJ):
                nc.scalar.copy(ot[:, j * NSPLIT:(j + 1) * NSPLIT], ps[j][:])
            nc.sync.dma_start(out=ctx_out[b:b + 1, :], in_=ot[0:1, :])

        for b in range(BP):
            nc.sync.dma_start(out=attn_out[b:b + 1, :, 0], in_=const_t[0:1, :])
            nc.sync.dma_start(out=cov_out[b:b + 1, :, 0], in_=const_t[1:2, :])

    nc.compile()
    return nc


def kernel(query=None, values=None, **unused_weights):
    from concourse.bass_utils import run_bass_kernel_spmd

    values = np.ascontiguousarray(np.asarray(values, dtype=np.float32))
    assert values.shape == (B, T, DV), values.shape

    if "nc" not in _CACHE:
        _CACHE["nc"] = _build_nc()
    nc = _CACHE["nc"]

    consts = np.stack(
        [np.ones(T, dtype=np.float32), np.arange(T, dtype=np.float32)]
    )
    core_ids = list(range(NCORES))
    in_maps = [
        {"vals": values[c * BP:(c + 1) * BP], "consts": consts}
        for c in core_ids
    ]

    trace = bool(int(os.environ.get("BASS_KERNEL_TRACE", "0")))
    res = run_bass_kernel_spmd(nc, in_maps, core_ids, trace=trace)
    LAST["exec_time_ns"] = res.exec_time_ns
    LAST["results"] = res

    context = np.concatenate([res.results[c]["ctx_out"] for c in core_ids], axis=0)
    attn = np.concatenate([res.results[c]["attn_out"] for c in core_ids], axis=0)
    coverage = np.concatenate([res.results[c]["cov_out"] for c in core_ids], axis=0)
    return context, attn, coverage
